# revision 3
# baseline (speedup 1.0000x reference)
"""Trainium2 Bass kernel for nn_ChannelSA3dCausal.

Computation (per batch b, time t):
  framed[c, t, d] = xpad[c, t+d]            (causal window, D=32)
  q/k = relu(BN(framed @ W^T + b))          (1x1 conv D->S=32, BN folded)
  ssa[i, j] = sum_s q[i,s] k[j,s] / sqrt(S)
  att[i] = sum_j softmax_j(ssa)[i,j] * x[j]
  out = x + att

Sharding: 8 cores = 4 batches x 2 time-halves (512 t each + 31-frame halo,
host-padded). Full inputs in, full output out.

Per-core pipeline (ScalarE-bound: 33.5M exp evals):
  - 4 "phases" g handle t = 128*g + tl; quad(tl) = 4 t's processed together.
  - framed SBUF tile [128, (c,tl)]: partition 32g+d holds x[c, 128g+tl+d] windows.
  - proj: ONE full-K matmul per q/k with host-built block-diagonal weights
    [128,128] -> all 4 phases at once; + fused DVE bias+relu evac -> fp32r.
  - ssa^T: per phase, 2 row-tiled fp32r matmuls (K=32 at tile_position (32g,0)),
    each concurrent tile in its own PSUM bank -> psum [j, i] per t.
  - exp: split ACT/DVE. ACT does cols [0,CA) exact (FD=CA, scale folded in,
    out bf16); DVE does cols [CA,1024) via Schraudolph: bf16 bits of exp(v*s)
    = round(v*s*log2e*128 + 16250.4) computed as one tensor_scalar mult+add
    with int16 output (HW converts round-to-nearest), bitcast to bf16.
    ~27% of E entries carry +-3.3% rel err; softmax num/den ratio cancels
    most of it (measured end-to-end ~1e-3).
    ReLU => ssa >= 0 => no max-subtraction needed (max scaled ssa ~17 << 88),
    and Schraudolph input is always >= 0 (no negative-int edge cases).
  - num/den: E^T as bf16 stationary [128,128], rhs = [x_t | 1] interleaved
    (host-built) -> psum [i, (num,den)] accumulating 128 t per bank.
    Issued in BURSTS covering `ndper` cycles (E lives in SBUF, so bursts
    only cost SBUF buffers): measured on HW, the PE pays ~350-400 ns at
    every junction where consecutive matmuls change dtype (fp32r<->bf16)
    or tile config (32-row tiled <-> full 128-row); bursting the bf16
    num/den amortizes its two junctions over `ndper` cycles instead of
    paying them every cycle.
  - divide + add x on DVE in [channel, t] layout; single output DMA.
"""

import sys

sys.path.insert(0, "/opt/trn_rl_repo")

import numpy as np

_KERNEL_CACHE = {}

B, C, T, D = 4, 256, 1024, 32
S = D
BN_EPS = 1e-5
TCORE = T // 2  # 512 t per core
HALO = D - 1  # 31
XSL_W = TCORE + HALO  # 543
NPH = 4  # phases
TPH = TCORE // NPH  # 128 t per phase
TBLK = 32  # tl per framed/nd block
NBLK = TPH // TBLK  # 4


def _build_program(reps=1, CA_=784, ndper=6):
    import concourse.bass as bass
    import concourse.bacc as bacc
    import concourse.tile as tile
    from concourse import mybir

    FP32 = mybir.dt.float32
    FP32R = mybir.dt.float32r
    BF16 = mybir.dt.bfloat16
    I16 = mybir.dt.int16
    AF = mybir.ActivationFunctionType
    ADD = mybir.AluOpType.add
    MAX = mybir.AluOpType.max
    MULT = mybir.AluOpType.mult

    nc = bacc.Bacc("TRN2", target_bir_lowering=False, debug=False)

    xsl_d = nc.dram_tensor("xsl", [C, XSL_W], FP32, kind="ExternalInput")
    # time-major rounded copy: framed DMA reads a contiguous 1KB c-row per
    # (partition, tls) descriptor instead of 256 4B strides (8x fewer descs)
    xslt_d = nc.dram_tensor("xslt", [XSL_W, C], FP32R, kind="ExternalInput")
    wblk_d = nc.dram_tensor("wblk", [128, 256], FP32R, kind="ExternalInput")
    bias_d = nc.dram_tensor("bias", [128, 2], FP32, kind="ExternalInput")
    xot_d = nc.dram_tensor("xot", [128, 4 * TCORE], BF16, kind="ExternalInput")
    out_d = nc.dram_tensor("out", [C, TCORE], FP32, kind="ExternalOutput")

    scale = float(1.0 / np.sqrt(np.float32(S)))
    # Schraudolph exp for the DVE share: bf16 bits = v*SCH_A + SCH_B,
    # computed fp32, converted to int16 (HW rounds to nearest), bitcast bf16.
    CA = CA_  # ACT handles cols [0, CA), DVE the remaining 1024-CA
    SCH_A = float(scale * 1.4426950408889634 * 128.0)
    SCH_B = 16250.4

    NITER = NBLK * TBLK  # 128 software-pipelined cycles per rep

    with tile.TileContext(nc) as tc:
        with (
            tc.tile_pool(name="persist", bufs=1) as pers,
            tc.tile_pool(name="framedp", bufs=2) as framedp,
            tc.tile_pool(name="qkp", bufs=3) as qkp,
            tc.tile_pool(name="ep", bufs=16) as ep,
            tc.tile_pool(name="divp", bufs=2) as divp,
            tc.tile_pool(name="pp", bufs=1, space="PSUM") as pp,
            tc.tile_pool(name="sp", bufs=3, space="PSUM") as sp,
            tc.tile_pool(name="ndp", bufs=1, space="PSUM") as ndp,
        ):
            # persistent tiles; DMAs are issued interleaved with the block-0
            # framed slices in the prologue (single DMA queue, ordered by
            # first-use time: framed slice 1, wblk, bias, xot, slices 2-3, xc)
            wblk = pers.tile([128, 256], FP32R, tag="wblk")
            biases = pers.tile([128, 2], FP32, tag="bias")
            xot = pers.tile([128, 4 * TCORE], BF16, tag="xot")
            xc = pers.tile([128, 2 * XSL_W], FP32, tag="xc")
            out_sb = pers.tile([128, 2 * TCORE], FP32, tag="osb")

            def load_persistents_early():
                nc.sync.dma_start(wblk[:], wblk_d.ap())
                nc.sync.dma_start(biases[:], bias_d.ap())
                nc.sync.dma_start(xot[:], xot_d.ap())

            def load_persistents_late():
                xc_dst = bass.AP(
                    tensor=xc[:].tensor,
                    offset=xc[:].offset,
                    ap=[[2 * XSL_W, 128], [XSL_W, 2], [1, XSL_W]],
                )
                nc.sync.dma_start(
                    xc_dst,
                    bass.AP(
                        tensor=xsl_d,
                        offset=0,
                        ap=[[XSL_W, 128], [128 * XSL_W, 2], [1, XSL_W]],
                    ),
                )

            # --- issue helpers (software-pipelined cycles) ---
            FRT = 16  # tls per framed range-tile; 2 tiles per block so proj
            # dependencies are precise (proj_0 starts ~15us in, not ~30us);
            # smaller slices fall below the 7ns/descriptor DMA floor

            def issue_framed_part(blk, part):
                # range tile: partition 32g+d, free col = (tls-part*FRT)*C + c
                # value = xsl[c, 128*g + TBLK*blk + tls + d]  (tls-major so the
                # inner DMA run is a contiguous C-row of the time-major source)
                fr = framedp.tile(
                    [128, C * FRT], FP32R, tag=f"fr{part}", name=f"fr{part}"
                )
                fr_dst = bass.AP(
                    tensor=fr[:].tensor,
                    offset=fr[:].offset,
                    ap=[[C * FRT, 128], [C, FRT], [1, C]],
                )
                fr_src = bass.AP(
                    tensor=xslt_d,
                    offset=(TPH * blk + FRT * part) * C,
                    ap=[[C, 128], [C, FRT], [1, C]],
                )
                nc.sync.dma_start(fr_dst, fr_src)
                return fr

            def issue_proj(m, framed):
                # PE: block-diag full-K matmuls (q, k) for iteration m
                tls = m % TBLK
                fr = framed[tls // FRT]
                prp = pp.tile([128, 512], FP32, tag="prp")
                rhs = bass.AP(
                    tensor=fr[:].tensor,
                    offset=fr[:].offset + (tls % FRT) * C,
                    ap=[[C * FRT, 128], [1, C]],
                )
                nc.tensor.matmul(
                    prp[:, 0:256], wblk[:, 0:128], rhs, start=True, stop=True
                )
                nc.tensor.matmul(
                    prp[:, 256:512], wblk[:, 128:256], rhs, start=True, stop=True
                )
                return prp

            def issue_evac(prp):
                # DVE: relu(h + bias) -> fp32r. High priority: the next
                # cycle's ssa waits on these, the Schraudolphs can wait.
                with tc.high_priority():
                    q_t = qkp.tile([128, 256], FP32R, tag="qt")
                    nc.vector.tensor_scalar(
                        q_t[:], prp[:, 0:256], biases[:, 0:1], 0.0, op0=ADD, op1=MAX
                    )
                    k_t = qkp.tile([128, 256], FP32R, tag="kt")
                    nc.vector.tensor_scalar(
                        k_t[:], prp[:, 256:512], biases[:, 1:2], 0.0, op0=ADD, op1=MAX
                    )
                return q_t, k_t

            def issue_ssa_half(h, q_t, k_t):
                # PE: ssa^T for phases g=2h, 2h+1 into one [128,1024] psum tile
                et = sp.tile([128, 1024], FP32, tag="e")
                for gg in range(2):
                    g = 2 * h + gg
                    for jc in range(2):
                        nc.tensor.matmul(
                            et[:, 512 * gg + 256 * jc : 512 * gg + 256 * (jc + 1)],
                            k_t[32 * g : 32 * g + 32, 128 * jc : 128 * (jc + 1)],
                            q_t[32 * g : 32 * g + 32, :],
                            start=True,
                            stop=True,
                            tile_position=(32 * g, 0),
                        )
                return et

            def issue_exp_act(et):
                # ACT exact exp on cols [0,CA)
                Et = ep.tile([128, 1024], BF16, tag="E")
                nc.scalar.activation(Et[:, 0:CA], et[:, 0:CA], AF.Exp, scale=scale)
                return Et

            def issue_exp_dve(et, Et):
                # DVE Schraudolph on [CA,1024); runs one cycle late so DVE's
                # stream (evac-q, evac-k, SchA, SchB) never head-blocks
                nc.vector.tensor_scalar(
                    Et[:, CA:1024].bitcast(I16),
                    et[:, CA:1024],
                    SCH_A,
                    SCH_B,
                    op0=MULT,
                    op1=ADD,
                )

            ndt_box = [None]

            def issue_numden(m, EA, EB):
                # PE: num/den for (delayed) iteration m; E^T stationary bf16
                blk, tls = divmod(m, TBLK)
                if tls == 0:
                    ndt_box[0] = ndp.tile(
                        [128, 512], FP32, tag="ndt", name="ndt"
                    )
                ndt = ndt_box[0]
                for g in range(4):
                    E_ = EA if g < 2 else EB
                    tp = TPH * blk + TBLK * g + tls  # t within core
                    colb = 16 * tls + 4 * g
                    for ic in range(2):
                        for jc in range(2):
                            nc.tensor.matmul(
                                ndt[:, colb + 2 * ic : colb + 2 * ic + 2],
                                E_[
                                    :,
                                    512 * (g % 2)
                                    + 256 * jc
                                    + 128 * ic : 512 * (g % 2)
                                    + 256 * jc
                                    + 128 * (ic + 1),
                                ],
                                xot[
                                    :, 2 * TCORE * jc + 2 * tp : 2 * TCORE * jc + 2 * tp + 2
                                ],
                                start=(jc == 0),
                                stop=(jc == 1),
                            )
                return ndt

            def issue_epilogue(blk, ndt):
                # DVE: evac nd, divide, add x for a finished block
                nd_sb = divp.tile([128, 512], FP32, tag="ndsb")
                nc.vector.tensor_copy(nd_sb[:], ndt[:])
                # cols: 16*tls + 4*g + 2*ic + e   (e: 0=num, 1=den)
                rden = divp.tile([128, 256], FP32, tag="rden")
                rden4 = bass.AP(
                    tensor=rden[:].tensor,
                    offset=rden[:].offset,
                    ap=[[256, 128], [8, TBLK], [2, NPH], [1, 2]],
                )
                den_ap = bass.AP(
                    tensor=nd_sb[:].tensor,
                    offset=nd_sb[:].offset + 1,
                    ap=[[512, 128], [16, TBLK], [4, NPH], [2, 2]],
                )
                nc.vector.reciprocal(rden4, den_ap)
                att = divp.tile([128, 256], FP32, tag="att")
                att4 = bass.AP(
                    tensor=att[:].tensor,
                    offset=att[:].offset,
                    ap=[[256, 128], [8, TBLK], [2, NPH], [1, 2]],
                )
                num_ap = bass.AP(
                    tensor=nd_sb[:].tensor,
                    offset=nd_sb[:].offset,
                    ap=[[512, 128], [16, TBLK], [4, NPH], [2, 2]],
                )
                nc.gpsimd.tensor_tensor(att4, num_ap, rden4, op=mybir.AluOpType.mult)
                # out_sb[p, ic*TCORE + 128*g + TBLK*blk + tls] = att + x
                out_ap = bass.AP(
                    tensor=out_sb[:].tensor,
                    offset=out_sb[:].offset + TPH * blk,
                    ap=[[2 * TCORE, 128], [1, TBLK], [TBLK, NPH], [TCORE, 2]],
                )
                x_ap = bass.AP(
                    tensor=xc[:].tensor,
                    offset=xc[:].offset + HALO + TPH * blk,
                    ap=[[2 * XSL_W, 128], [1, TBLK], [TBLK, NPH], [XSL_W, 2]],
                )
                nc.gpsimd.tensor_tensor(out_ap, att4, x_ap, op=ADD)
                # stream this block's slice of the output back to DRAM
                osb_src = bass.AP(
                    tensor=out_sb[:].tensor,
                    offset=out_sb[:].offset + TPH * blk,
                    ap=[[2 * TCORE, 128], [TCORE, 2], [1, TPH]],
                )
                nc.sync.dma_start(
                    bass.AP(
                        tensor=out_d,
                        offset=TPH * blk,
                        ap=[[TCORE, 128], [128 * TCORE, 2], [1, TPH]],
                    ),
                    osb_src,
                )

            for _rep in range(reps):
                # prologue: block-0 framed range-tiles streamed in first-use
                # order with persistent loads interleaved (single DMA queue);
                # proj_0, evac_0, proj_1 so cycle 0 can start with evac_1 at
                # the front of DVE's stream
                fr0 = issue_framed_part(0, 0)
                load_persistents_early()
                framed_cur = [fr0, None]
                qk_cur = issue_evac(issue_proj(0, framed_cur))
                prp_pend = issue_proj(1, framed_cur)  # evac'd at cycle 0 front
                framed_cur[1] = issue_framed_part(0, 1)
                load_persistents_late()
                framed_next = None
                qk_next = None
                E_q = {}
                nd_next = 0

                # cycle m (steady state, every engine stall-free):
                #   DVE: evac_{m+1} (prp from cycle m-1), dve-expA_m, dve-expB_m
                #   PE : ssa-A_m, ssa-B_m, proj_{m+2}, numden_{m-1}
                #   ACT: expA_m, expB_m
                eE_prev = None  # (eA, eB, EA, EB) of cycle m-1: Sch pending

                for m in range(NITER):
                    blk, tls = divmod(m, TBLK)
                    if tls == 0 and blk + 1 < NBLK:
                        framed_next = [
                            issue_framed_part(blk + 1, part) for part in range(2)
                        ]
                    if m + 1 < NITER:
                        qk_next = issue_evac(prp_pend)
                    if eE_prev is not None:
                        issue_exp_dve(eE_prev[0], eE_prev[2])
                        issue_exp_dve(eE_prev[1], eE_prev[3])
                    q_t, k_t = qk_cur
                    eA = issue_ssa_half(0, q_t, k_t)
                    eB = issue_ssa_half(1, q_t, k_t)
                    EA = issue_exp_act(eA)
                    EB = issue_exp_act(eB)
                    if m + 2 < NITER:
                        fr = framed_next if (m + 2) // TBLK > blk else framed_cur
                        prp_pend = issue_proj(m + 2, fr)
                    if eE_prev is not None:
                        E_q[m - 1] = (eE_prev[2], eE_prev[3])
                    if m % ndper == ndper - 1:
                        for c in range(nd_next, m - 1):
                            ndt = issue_numden(c, *E_q.pop(c))
                            if c % TBLK == TBLK - 1:
                                issue_epilogue(c // TBLK, ndt)
                        nd_next = m - 1
                    eE_prev = (eA, eB, EA, EB)
                    qk_cur = qk_next
                    if tls == TBLK - 1:
                        framed_cur = framed_next

                # drain: Schraudolph for the final cycle, then the remaining
                # num/dens + epilogues (which stream the output)
                issue_exp_dve(eE_prev[0], eE_prev[2])
                issue_exp_dve(eE_prev[1], eE_prev[3])
                E_q[NITER - 1] = (eE_prev[2], eE_prev[3])
                for c in range(nd_next, NITER):
                    ndt = issue_numden(c, *E_q.pop(c))
                    if c % TBLK == TBLK - 1:
                        issue_epilogue(c // TBLK, ndt)

    nc.compile()
    return nc


def _host_prep(inputs):
    """Fold BN into weights, build per-core input maps."""
    x = np.asarray(inputs["x"], dtype=np.float32)  # [B, C, T, 1]
    xs = x[..., 0]  # [B, C, T]

    def fold(w, b, gamma, beta):
        g = np.asarray(gamma, np.float32) / np.sqrt(np.float32(1.0 + BN_EPS))
        wp = np.asarray(w, np.float32) * g[:, None]  # [s, d]
        bp = np.asarray(b, np.float32) * g + np.asarray(beta, np.float32)
        return wp, bp

    def round_fp32r(a):
        # approximate the fp32r operand rounding (~13-14 mantissa bits kept);
        # the PE truncates further internally either way.
        u = np.ascontiguousarray(a, np.float32).view(np.uint32)
        u = (u + np.uint32(0x100)) & np.uint32(0xFFFFFE00)
        return u.view(np.float32)

    wq, bq = fold(
        inputs["query_w"], inputs["query_b"], inputs["query_gamma"], inputs["query_beta"]
    )
    wk, bk = fold(
        inputs["key_w"], inputs["key_b"], inputs["key_gamma"], inputs["key_beta"]
    )

    # block-diag weights [128, 256]: [:, 0:128]=q, [:, 128:256]=k
    # wblk[32g+d, 32g+s] = w[s, d]
    wblk = np.zeros((128, 256), np.float32)
    for g in range(NPH):
        wblk[32 * g : 32 * g + 32, 32 * g : 32 * g + 32] = wq.T
        wblk[32 * g : 32 * g + 32, 128 + 32 * g : 128 + 32 * g + 32] = wk.T
    bias2 = np.zeros((128, 2), np.float32)
    bias2[:, 0] = np.tile(bq, NPH)
    bias2[:, 1] = np.tile(bk, NPH)

    xpad = np.concatenate([np.zeros((B, C, HALO), np.float32), xs], axis=2)

    import ml_dtypes

    in_maps = []
    for core in range(8):
        b, th = core // 2, core % 2
        t0 = th * TCORE
        xsl = np.ascontiguousarray(xpad[b, :, t0 : t0 + XSL_W])  # [C, 543]
        xslt = round_fp32r(np.ascontiguousarray(xsl.T))  # [543, C] time-major
        # xot [128, 4*TCORE] bf16: [p, 2*TCORE*jc + 2*t' + e]
        xot = np.ones((128, 4 * TCORE), np.float32)
        for jc in range(2):
            xot[:, 2 * TCORE * jc : 2 * TCORE * (jc + 1) : 2] = xs[
                b, 128 * jc : 128 * (jc + 1), t0 : t0 + TCORE
            ]
        in_maps.append(
            {
                "xsl": xsl,
                "xslt": xslt,
                "wblk": round_fp32r(wblk),
                "bias": bias2,
                "xot": xot.astype(ml_dtypes.bfloat16),
            }
        )
    return in_maps


def kernel(**inputs):
    from concourse.bass_utils import run_bass_kernel_spmd

    if "nc" not in _KERNEL_CACHE:
        _KERNEL_CACHE["nc"] = _build_program()
    nc = _KERNEL_CACHE["nc"]

    in_maps = _host_prep(inputs)
    res = run_bass_kernel_spmd(nc, in_maps, core_ids=list(range(8)))
    _KERNEL_CACHE["last_results"] = res

    x = np.asarray(inputs["x"], dtype=np.float32)
    out = np.empty((B, C, T, 1), dtype=np.float32)
    for core in range(8):
        b, th = core // 2, core % 2
        t0 = th * TCORE
        out[b, :, t0 : t0 + TCORE, 0] = res.results[core]["out"]
    return out

